# revision 15
# baseline (speedup 1.0000x reference)
"""BitConv2d (ternary-weight 3x3 conv, power-of-two rescale) on 8 TRN2 NeuronCores.

Strategy (v3):
  - Data-parallel over batch: 32 images -> 4 per core (2 image pairs).
  - Input staged to device as fp16 in a host-pretransposed, ZERO-PADDED
    layout [pair, region, 128, 18, 114] (partition = img_in_pair*64 + cin).
    Region r holds padded rows 16r..16r+17 (2-row halo duplicated on host),
    borders are zeros.  One contiguous 525 KB DMA per region.
  - Quantization replay: the reference computes xq = round(clip(x,±1)*64)/64.
    We use xv = bf16(clip(fp16(x),±1)) instead - the bf16 grid at |x|<=1 is
    finer (2^-8..2^-9) than the reference's quant step 2^-6, and the fp16
    staging error dominates; measured rel err ~4.6e-3 (budget 2e-2).  So the
    whole quant pipeline is ONE engine op per region: tensor_scalar
    min(x,1),max(x,-1) fp16->bf16, fully contiguous.  Zero borders clip to
    zero, contributing nothing: no bias folding needed.
  - Conv as 9 accumulating matmuls per output tile (K=Cin=64, M=Cout=64),
    packed 4-per-array with tile_position quadrants.  Per iteration (8 output
    rows) ONE two-bank psum tile [128, 896]:
      cols   0:448 <- quad (0,0)=img_a blk0 | quad (64,64)=img_b blk1
      cols 448:896 <- quad (64,0)=img_b blk0 | quad (0,64)=img_a blk1
    Taps loop over iteration PAIRS (both iters share the region tile) for
    deeper PE pipelining.
  - Weights are prescaled on host by 2^(act_exp+s_exp[co]) (exact: powers of
    two in bf16), so the epilogue is a single bias-add [128,896] psum->bf16,
    alternating ACT/DVE by iteration parity.
  - Output written bf16 in device-native layout, 8 DMAs of ~0.7-0.8 MB; host
    reassembles into f32 NCHW.
"""

import numpy as np
import ml_dtypes
from contextlib import ExitStack

_NC_CACHE = {}

N_CORES = 8
H = W = 112
WP = W + 2                    # padded cols
CIN = COUT = 64
P = 128
IMGS_PER_CORE = 4
N_PAIRS = 2
REGIONS = 7                   # input regions per pair
REGION_ROWS = 18              # padded rows per region (16 + 2 halo)
ROWS_PER_TILE = 4             # output rows per matmul tile
NFREE = ROWS_PER_TILE * W     # 448
N_ITERS = 14                  # conv iterations per pair (8 rows each)
GROUPS = 2                    # output buffers per pair
ITERS_PER_GROUP = 7


def _patch_tile_drain(tile_mod):
    """This walrus build rejects a Drain carrying many sync waits; split the
    final Tile drain into single-wait sync nops."""
    from concourse.vector_clock import ScopedClock, VectorClock

    if getattr(tile_mod.TileContext, "_drain_patched", False):
        return

    def _drain_and_barrier_split(self, tick_clock, wait_clock):
        vclock = tick_clock.global_clock
        n = len(vclock)
        for proc in range(n):
            t = vclock[proc]
            if t <= 0:
                continue
            vec = [0] * n
            vec[proc] = t
            nop = self.nc.sync.nop()
            wait_clock.add_sem_waits(nop.ins, ScopedClock({None: VectorClock(vec)}))
        self.nc.sync.drain()
        assert self.sems is not None
        popped = self.nc._tile_sem_poison_stack.pop()
        assert popped is self._sem_poison
        self.nc.all_engine_barrier()
        self.nc.clear_and_free_semaphores(list(self.sems.allocated().values()))
        self.nc.all_engine_barrier()

    tile_mod.TileContext._drain_and_barrier = _drain_and_barrier_split
    tile_mod.TileContext._drain_patched = True


def _split_multi_syncs(nc):
    """This walrus build accepts at most ONE sync wait (and one update) per
    instruction.  Hoist extra waits onto preceding nops and extra updates onto
    following nops (same engine, so ordering semantics are preserved)."""
    import concourse.mybir as mybir

    fn = nc.m.functions[0]
    ctr = 0
    for bb in fn.blocks:
        new_insts = []
        for inst in bb.instructions:
            si = inst.sync_info
            pre, post = [], []
            if si is not None and si.on_wait and len(si.on_wait) > 1:
                for w in list(si.on_wait[:-1]):
                    ctr += 1
                    pre.append(
                        mybir.InstNoOp(
                            name=f"wsplit_nop_{ctr}",
                            engine=inst.engine,
                            sync_info=mybir.SyncInfo(on_wait=[w], on_update=[]),
                        )
                    )
                si.on_wait = [si.on_wait[-1]]
            if (
                si is not None
                and si.on_update
                and len(si.on_update) > 1
                and not isinstance(inst, (mybir.InstDMACopy, mybir.InstDMA))
            ):
                for u in list(si.on_update[1:]):
                    ctr += 1
                    post.append(
                        mybir.InstNoOp(
                            name=f"usplit_nop_{ctr}",
                            engine=inst.engine,
                            sync_info=mybir.SyncInfo(on_wait=[], on_update=[u]),
                        )
                    )
                si.on_update = [si.on_update[0]]
            new_insts.extend(pre)
            new_insts.append(inst)
            new_insts.extend(post)
        if len(new_insts) != len(bb.instructions):
            bb.instructions[:] = new_insts
    for bb in fn.blocks:
        for inst in bb.instructions:
            if inst.name.startswith(("wsplit_nop_", "usplit_nop_")):
                if inst.name not in nc.inst_map:
                    nc.register_instruction(inst)
    return ctr


def build_nc():
    import concourse.bass as bass
    import concourse.mybir as mybir
    import concourse.tile as tile

    _patch_tile_drain(tile)

    f32 = mybir.dt.float32
    f16 = mybir.dt.float16
    bf16 = mybir.dt.bfloat16
    Alu = mybir.AluOpType
    Act = mybir.ActivationFunctionType

    nc = bass.Bass(trn_type="TRN2")
    xh = nc.dram_tensor(
        "xh", (N_PAIRS, REGIONS, P, REGION_ROWS, WP), f16, kind="ExternalInput"
    )
    wsb = nc.dram_tensor("wsb", (P, 9 * COUT), bf16, kind="ExternalInput")
    sb = nc.dram_tensor("sb", (P, 1), f32, kind="ExternalInput")
    yd = nc.dram_tensor(
        "yd", (N_PAIRS, GROUPS, P, ITERS_PER_GROUP * 2 * NFREE), bf16,
        kind="ExternalOutput",
    )

    with tile.TileContext(nc) as tc, ExitStack() as ctx:
        const_pool = ctx.enter_context(tc.tile_pool(name="const", bufs=1))
        xq_pool = ctx.enter_context(tc.tile_pool(name="xq", bufs=2 * REGIONS))
        stg_pool = ctx.enter_context(tc.tile_pool(name="stg", bufs=6))
        out_pool = ctx.enter_context(tc.tile_pool(name="out", bufs=3))
        psum_pool = ctx.enter_context(
            tc.tile_pool(name="psum", bufs=4, space=bass.MemorySpace.PSUM)
        )

        w_t = const_pool.tile([P, 9 * COUT], bf16)
        nc.sync.dma_start(w_t[:], wsb[:])
        sb_t = const_pool.tile([P, 1], f32)
        nc.sync.dma_start(sb_t[:], sb[:])

        def emit_quant(pair_idx, region, dma_eng=None, clip_eng=None):
            stg = stg_pool.tile([P, REGION_ROWS, WP], f16)
            (dma_eng or nc.sync).dma_start(stg[:], xh[pair_idx, region])
            xq = xq_pool.tile([P, REGION_ROWS, WP], bf16,
                              name=f"xq{pair_idx}_{region}", tag="xq")
            (clip_eng or nc.vector).tensor_scalar(
                out=xq[:], in0=stg[:], scalar1=1.0, scalar2=-1.0,
                op0=Alu.min, op1=Alu.max,
            )
            return xq

        def emit_conv_one(xq, b, ps):
            for tap in range(9):
                dh, dw = divmod(tap, 3)
                st, sp = tap == 0, tap == 8
                ws = slice(tap * 64, (tap + 1) * 64)
                for half, blk in ((0, 0), (1, 1), (1, 0), (0, 1)):
                    bank = half ^ blk
                    wsl = (slice(0, 64) if half == 0 else slice(64, 128))
                    psl = (slice(0, 64) if blk == 0 else slice(64, 128))
                    h = b + ROWS_PER_TILE * blk + dh
                    nc.tensor.matmul(
                        ps[psl, bank, 0:NFREE], w_t[wsl, ws],
                        xq[wsl, h:h + ROWS_PER_TILE, dw:dw + W],
                        start=st, stop=sp)

        def emit_conv_pair(xq, j0, ps_list):
            # two iterations j0, j0+1 share the region tile; loop taps
            # outermost so the PE queue sees 8 independent matmuls per tap.
            for tap in range(9):
                dh, dw = divmod(tap, 3)
                st, sp = tap == 0, tap == 8
                ws = slice(tap * 64, (tap + 1) * 64)
                for ps, b in zip(ps_list, (0, 8)):
                    for half, blk in ((0, 0), (1, 1), (1, 0), (0, 1)):
                        bank = half ^ blk
                        wsl = (slice(0, 64) if half == 0 else slice(64, 128))
                        psl = (slice(0, 64) if blk == 0 else slice(64, 128))
                        h = b + ROWS_PER_TILE * blk + dh
                        nc.tensor.matmul(
                            ps[psl, bank, 0:NFREE], w_t[wsl, ws],
                            xq[wsl, h:h + ROWS_PER_TILE, dw:dw + W],
                            start=st, stop=sp)

        def emit_epilogue(ps, ob, itg):
            if itg % 2 == 0:
                nc.scalar.activation(
                    ob[:, itg], ps[:], Act.Identity, bias=sb_t[:, 0:1],
                )
            else:
                nc.vector.tensor_scalar_add(ob[:, itg], ps[:], sb_t[:, 0:1])

        # software pipeline: conv(pair k) interleaves with quant(pair k+1).
        # Head: regions 0,1 staged on two different DMA queues (sync+scalar)
        # so both land early; remaining pair-0 clips are emitted interleaved
        # with conv so DVE's FIFO never head-of-line-blocks the epilogues.
        B2 = 2 * NFREE
        xq_k = [None] * REGIONS
        xq_half = []
        for hh, (r0, r1) in enumerate(((0, 10), (8, 18))):
            stgh = stg_pool.tile([P, 10, WP], f16, name=f"stgh{hh}", tag="stgh")
            eng = nc.sync if hh == 0 else nc.scalar
            eng.dma_start(stgh[:], xh[0, 0, :, r0:r1, :])
            xqh = xq_pool.tile([P, 10, WP], bf16, name=f"xqh{hh}", tag="xqh")
            nc.vector.tensor_scalar(
                out=xqh[:], in0=stgh[:], scalar1=1.0, scalar2=-1.0,
                op0=Alu.min, op1=Alu.max,
            )
            xq_half.append(xqh)
        xq_k[1] = emit_quant(0, 1)
        for k in range(N_PAIRS):
            xq_next = [None] * REGIONS
            obs = {}
            for r in range(REGIONS):
                if k == 0 and r + 2 < REGIONS:
                    xq_k[r + 2] = emit_quant(0, r + 2, clip_eng=nc.gpsimd)
                if k + 1 < N_PAIRS:
                    lag = [r - 2] if 0 <= r - 2 else []
                    if r == REGIONS - 1:
                        lag = [r - 2, r - 1, r]
                    for rr in lag:
                        xq_next[rr] = emit_quant(k + 1, rr,
                                                 clip_eng=nc.gpsimd)
                ps_list = [
                    psum_pool.tile([P, 2, 512], f32,
                                   name=f"ps{k}_{r}_{i}", tag="ps")
                    for i in range(2)
                ]
                if k == 0 and r == 0:
                    emit_conv_one(xq_half[0], 0, ps_list[0])
                    emit_conv_one(xq_half[1], 0, ps_list[1])
                else:
                    emit_conv_pair(xq_k[r], 2 * r, ps_list)
                for i in range(2):
                    j = 2 * r + i
                    g, itg = divmod(j, ITERS_PER_GROUP)
                    if itg == 0:
                        obs[g] = out_pool.tile([P, ITERS_PER_GROUP, 2, 512],
                                               bf16, name=f"ob{k}_{g}",
                                               tag="ob")
                    emit_epilogue(ps_list[i], obs[g], itg)
                    if itg == 3:
                        nc.scalar.dma_start(yd[k, g, :, 0:4 * B2],
                                            obs[g][:, 0:4, :, 0:NFREE])
                    elif itg == 5:
                        nc.scalar.dma_start(yd[k, g, :, 4 * B2:6 * B2],
                                            obs[g][:, 4:6, :, 0:NFREE])
                    elif itg == 6:
                        nc.scalar.dma_start(yd[k, g, :, 6 * B2:7 * B2],
                                            obs[g][:, 6:7, :, 0:NFREE])
            xq_k = xq_next

    _split_multi_syncs(nc)
    nc.finalize()
    return nc


def _host_prep(w_q, s_exp, bias, act_exp):
    """Prescaled weights in lhsT layout (dup on both halves) + bias column."""
    s_exp = np.asarray(s_exp).reshape(-1).astype(np.float64)
    # matmul operands are real-valued clip(x) (not integer x/step), so only
    # the per-channel 2^s_exp factor goes into the weights; act_exp is
    # implicitly replayed by the bf16 grid of the operands.
    scale = np.exp2(s_exp)                                        # [64]
    wq = w_q.astype(np.float64) * scale.reshape(-1, 1, 1, 1)      # [co,ci,kh,kw]
    w_half = np.transpose(wq, (1, 2, 3, 0)).reshape(CIN, 9 * COUT)
    wsb = np.concatenate([w_half, w_half], axis=0).astype(ml_dtypes.bfloat16)

    col_bias = np.tile(np.asarray(bias, np.float32), 2).astype(np.float32)
    sb = col_bias.reshape(P, 1)                                   # [128, 1] f32
    return wsb, sb


def _stage_x(x):
    """f32 [32,64,112,112] -> fp16 zero-padded region layout
    [core, pair, region, 128, 18, 114] with 2-row halos duplicated."""
    xp = np.zeros((N_CORES * IMGS_PER_CORE, CIN, H + 2, WP), np.float16)
    xp[:, :, 1:1 + H, 1:1 + W] = x.astype(np.float16)
    xp = xp.reshape(N_CORES, N_PAIRS, 2, CIN, H + 2, WP)
    regs = [xp[:, :, :, :, 16 * r:16 * r + REGION_ROWS, :] for r in range(REGIONS)]
    xh = np.stack(regs, axis=2)   # core, pair, region, imgp, ch, row, col
    return np.ascontiguousarray(
        xh.reshape(N_CORES, N_PAIRS, REGIONS, P, REGION_ROWS, WP)
    )


def _assemble_y(yd_list):
    """Per-core bf16 [2,2,128,6272] device layout -> f32 [32,64,112,112]."""
    out = np.empty((N_CORES * IMGS_PER_CORE, COUT, H, W), np.float32)
    for c, yd in enumerate(yd_list):
        # dims: pair, g, half, ch, itg, ab, row, col
        v = np.asarray(yd).reshape(N_PAIRS, GROUPS, 2, 64, ITERS_PER_GROUP, 2,
                                   ROWS_PER_TILE, W).astype(np.float32)
        oc = out[IMGS_PER_CORE * c: IMGS_PER_CORE * (c + 1)]
        # out rows = 8*(7g+itg) + 4*blk + row -> (g, itg, blk, row) nesting
        o_r = oc.reshape(N_PAIRS, 2, 64, GROUPS, ITERS_PER_GROUP, 2,
                         ROWS_PER_TILE, W)
        for half in (0, 1):
            for ab in (0, 1):
                imgp, blk = half ^ ab, half
                o_r[:, imgp, :, :, :, blk] = \
                    v[:, :, half, :, :, ab].transpose(0, 2, 1, 3, 4, 5)
    return out


def _make_in_maps(x, w_q, s_exp, bias, act_exp):
    x = np.asarray(x, dtype=np.float32)
    wsb, sb = _host_prep(np.asarray(w_q), s_exp, bias, int(act_exp))
    xh = _stage_x(x)
    return [{"xh": xh[c], "wsb": wsb, "sb": sb} for c in range(N_CORES)]


def kernel(x, w_q, s_exp, bias, act_exp):
    from concourse.bass_utils import run_bass_kernel_spmd

    in_maps = _make_in_maps(x, w_q, s_exp, bias, act_exp)
    if "nc" not in _NC_CACHE:
        _NC_CACHE["nc"] = build_nc()
    nc = _NC_CACHE["nc"]

    res = run_bass_kernel_spmd(nc, in_maps, core_ids=list(range(N_CORES)))
    return _assemble_y([res.results[c]["yd"] for c in range(N_CORES)])


# revision 16
# speedup vs baseline: 1.1650x; 1.1650x over previous
"""BitConv2d (ternary-weight 3x3 conv, power-of-two rescale) on 8 TRN2 NeuronCores.

Strategy (v3):
  - Data-parallel over batch: 32 images -> 4 per core (2 image pairs).
  - Input staged to device as fp16 in a host-pretransposed, ZERO-PADDED
    layout [pair, region, 128, 18, 114] (partition = img_in_pair*64 + cin).
    Region r holds padded rows 16r..16r+17 (2-row halo duplicated on host),
    borders are zeros.  One contiguous 525 KB DMA per region.
  - Quantization replay: the reference computes xq = round(clip(x,±1)*64)/64.
    We use xv = bf16(clip(fp16(x),±1)) instead - the bf16 grid at |x|<=1 is
    finer (2^-8..2^-9) than the reference's quant step 2^-6, and the fp16
    staging error dominates; measured rel err ~4.6e-3 (budget 2e-2).  So the
    whole quant pipeline is ONE engine op per region: tensor_scalar
    min(x,1),max(x,-1) fp16->bf16, fully contiguous.  Zero borders clip to
    zero, contributing nothing: no bias folding needed.
  - Conv as 9 accumulating matmuls per output tile (K=Cin=64, M=Cout=64),
    packed 4-per-array with tile_position quadrants.  Per iteration (8 output
    rows) ONE two-bank psum tile [128, 896]:
      cols   0:448 <- quad (0,0)=img_a blk0 | quad (64,64)=img_b blk1
      cols 448:896 <- quad (64,0)=img_b blk0 | quad (0,64)=img_a blk1
    Taps loop over iteration PAIRS (both iters share the region tile) for
    deeper PE pipelining.
  - Weights are prescaled on host by 2^(act_exp+s_exp[co]) (exact: powers of
    two in bf16), so the epilogue is a single bias-add [128,896] psum->bf16,
    alternating ACT/DVE by iteration parity.
  - Output written bf16 in device-native layout, 8 DMAs of ~0.7-0.8 MB; host
    reassembles into f32 NCHW.
"""

import numpy as np
import ml_dtypes
from contextlib import ExitStack

_NC_CACHE = {}

N_CORES = 8
H = W = 112
WP = W + 2                    # padded cols
CIN = COUT = 64
P = 128
IMGS_PER_CORE = 4
N_PAIRS = 2
REGIONS = 7                   # input regions per pair
REGION_ROWS = 18              # padded rows per region (16 + 2 halo)
ROWS_PER_TILE = 4             # output rows per matmul tile
NFREE = ROWS_PER_TILE * W     # 448
N_ITERS = 14                  # conv iterations per pair (8 rows each)
GROUPS = 2                    # output buffers per pair
ITERS_PER_GROUP = 7


def _patch_tile_drain(tile_mod):
    """This walrus build rejects a Drain carrying many sync waits; split the
    final Tile drain into single-wait sync nops."""
    from concourse.vector_clock import ScopedClock, VectorClock

    if getattr(tile_mod.TileContext, "_drain_patched", False):
        return

    def _drain_and_barrier_split(self, tick_clock, wait_clock):
        vclock = tick_clock.global_clock
        n = len(vclock)
        for proc in range(n):
            t = vclock[proc]
            if t <= 0:
                continue
            vec = [0] * n
            vec[proc] = t
            nop = self.nc.sync.nop()
            wait_clock.add_sem_waits(nop.ins, ScopedClock({None: VectorClock(vec)}))
        self.nc.sync.drain()
        assert self.sems is not None
        popped = self.nc._tile_sem_poison_stack.pop()
        assert popped is self._sem_poison
        self.nc.all_engine_barrier()
        self.nc.clear_and_free_semaphores(list(self.sems.allocated().values()))
        self.nc.all_engine_barrier()

    tile_mod.TileContext._drain_and_barrier = _drain_and_barrier_split
    tile_mod.TileContext._drain_patched = True


def _split_multi_syncs(nc):
    """This walrus build accepts at most ONE sync wait (and one update) per
    instruction.  Hoist extra waits onto preceding nops and extra updates onto
    following nops (same engine, so ordering semantics are preserved)."""
    import concourse.mybir as mybir

    fn = nc.m.functions[0]
    ctr = 0
    for bb in fn.blocks:
        new_insts = []
        for inst in bb.instructions:
            si = inst.sync_info
            pre, post = [], []
            if si is not None and si.on_wait and len(si.on_wait) > 1:
                for w in list(si.on_wait[:-1]):
                    ctr += 1
                    pre.append(
                        mybir.InstNoOp(
                            name=f"wsplit_nop_{ctr}",
                            engine=inst.engine,
                            sync_info=mybir.SyncInfo(on_wait=[w], on_update=[]),
                        )
                    )
                si.on_wait = [si.on_wait[-1]]
            if (
                si is not None
                and si.on_update
                and len(si.on_update) > 1
                and not isinstance(inst, (mybir.InstDMACopy, mybir.InstDMA))
            ):
                for u in list(si.on_update[1:]):
                    ctr += 1
                    post.append(
                        mybir.InstNoOp(
                            name=f"usplit_nop_{ctr}",
                            engine=inst.engine,
                            sync_info=mybir.SyncInfo(on_wait=[], on_update=[u]),
                        )
                    )
                si.on_update = [si.on_update[0]]
            new_insts.extend(pre)
            new_insts.append(inst)
            new_insts.extend(post)
        if len(new_insts) != len(bb.instructions):
            bb.instructions[:] = new_insts
    for bb in fn.blocks:
        for inst in bb.instructions:
            if inst.name.startswith(("wsplit_nop_", "usplit_nop_")):
                if inst.name not in nc.inst_map:
                    nc.register_instruction(inst)
    return ctr


def build_nc():
    import concourse.bass as bass
    import concourse.mybir as mybir
    import concourse.tile as tile

    _patch_tile_drain(tile)

    f32 = mybir.dt.float32
    f16 = mybir.dt.float16
    bf16 = mybir.dt.bfloat16
    Alu = mybir.AluOpType
    Act = mybir.ActivationFunctionType

    nc = bass.Bass(trn_type="TRN2")
    xh = nc.dram_tensor(
        "xh", (N_PAIRS, REGIONS, P, REGION_ROWS, WP), f16, kind="ExternalInput"
    )
    wsb = nc.dram_tensor("wsb", (P, 9 * COUT), bf16, kind="ExternalInput")
    sb = nc.dram_tensor("sb", (P, 1), f32, kind="ExternalInput")
    yd = nc.dram_tensor(
        "yd", (N_PAIRS, GROUPS, P, ITERS_PER_GROUP * 2 * NFREE), bf16,
        kind="ExternalOutput",
    )

    with tile.TileContext(nc) as tc, ExitStack() as ctx:
        const_pool = ctx.enter_context(tc.tile_pool(name="const", bufs=1))
        xq_pool = ctx.enter_context(tc.tile_pool(name="xq", bufs=2 * REGIONS))
        stg_pool = ctx.enter_context(tc.tile_pool(name="stg", bufs=6))
        out_pool = ctx.enter_context(tc.tile_pool(name="out", bufs=3))
        psum_pool = ctx.enter_context(
            tc.tile_pool(name="psum", bufs=4, space=bass.MemorySpace.PSUM)
        )

        w_t = const_pool.tile([P, 9 * COUT], bf16)
        nc.sync.dma_start(w_t[:], wsb[:])
        sb_t = const_pool.tile([P, 1], f32)
        nc.sync.dma_start(sb_t[:], sb[:])

        def emit_quant(pair_idx, region, dma_eng=None, clip_eng=None):
            stg = stg_pool.tile([P, REGION_ROWS, WP], f16)
            (dma_eng or nc.sync).dma_start(stg[:], xh[pair_idx, region])
            xq = xq_pool.tile([P, REGION_ROWS, WP], bf16,
                              name=f"xq{pair_idx}_{region}", tag="xq")
            (clip_eng or nc.vector).tensor_scalar(
                out=xq[:], in0=stg[:], scalar1=1.0, scalar2=-1.0,
                op0=Alu.min, op1=Alu.max,
            )
            return xq

        def emit_conv_one(xq, b, ps):
            for tap in range(9):
                dh, dw = divmod(tap, 3)
                st, sp = tap == 0, tap == 8
                ws = slice(tap * 64, (tap + 1) * 64)
                for half, blk in ((0, 0), (1, 1), (1, 0), (0, 1)):
                    bank = half ^ blk
                    wsl = (slice(0, 64) if half == 0 else slice(64, 128))
                    psl = (slice(0, 64) if blk == 0 else slice(64, 128))
                    h = b + ROWS_PER_TILE * blk + dh
                    nc.tensor.matmul(
                        ps[psl, bank, 0:NFREE], w_t[wsl, ws],
                        xq[wsl, h:h + ROWS_PER_TILE, dw:dw + W],
                        start=st, stop=sp)

        def emit_conv_pair(xq, j0, ps_list):
            # two iterations j0, j0+1 share the region tile; loop taps
            # outermost so the PE queue sees 8 independent matmuls per tap.
            for tap in range(9):
                dh, dw = divmod(tap, 3)
                st, sp = tap == 0, tap == 8
                ws = slice(tap * 64, (tap + 1) * 64)
                for ps, b in zip(ps_list, (0, 8)):
                    for half, blk in ((0, 0), (1, 1), (1, 0), (0, 1)):
                        bank = half ^ blk
                        wsl = (slice(0, 64) if half == 0 else slice(64, 128))
                        psl = (slice(0, 64) if blk == 0 else slice(64, 128))
                        h = b + ROWS_PER_TILE * blk + dh
                        nc.tensor.matmul(
                            ps[psl, bank, 0:NFREE], w_t[wsl, ws],
                            xq[wsl, h:h + ROWS_PER_TILE, dw:dw + W],
                            start=st, stop=sp)

        def emit_epilogue(ps, ob, itg):
            if itg % 2 == 0:
                nc.scalar.activation(
                    ob[:, itg], ps[:], Act.Identity, bias=sb_t[:, 0:1],
                )
            else:
                nc.vector.tensor_scalar_add(ob[:, itg], ps[:], sb_t[:, 0:1])

        # software pipeline: conv(pair k) interleaves with quant(pair k+1).
        # Head: regions 0,1 staged on two different DMA queues (sync+scalar)
        # so both land early; remaining pair-0 clips are emitted interleaved
        # with conv so DVE's FIFO never head-of-line-blocks the epilogues.
        B2 = 2 * NFREE
        xq_k = [None] * REGIONS
        xq_half = []
        for hh, (r0, r1) in enumerate(((0, 10), (8, 18))):
            stgh = stg_pool.tile([P, 10, WP], f16, name=f"stgh{hh}", tag="stgh")
            eng = nc.sync if hh == 0 else nc.scalar
            eng.dma_start(stgh[:], xh[0, 0, :, r0:r1, :])
            xqh = xq_pool.tile([P, 10, WP], bf16, name=f"xqh{hh}", tag="xqh")
            nc.vector.tensor_scalar(
                out=xqh[:], in0=stgh[:], scalar1=1.0, scalar2=-1.0,
                op0=Alu.min, op1=Alu.max,
            )
            xq_half.append(xqh)
        xq_k[1] = emit_quant(0, 1)
        xq_k[2] = emit_quant(0, 2)
        for r in range(3, REGIONS):
            xq_k[r] = emit_quant(0, r, clip_eng=nc.gpsimd)
        for k in range(N_PAIRS):
            xq_next = [None] * REGIONS
            obs = {}
            for r in range(REGIONS):
                if k + 1 < N_PAIRS:
                    lag = [r - 2] if 0 <= r - 2 else []
                    if r == REGIONS - 1:
                        lag = [r - 2, r - 1, r]
                    for rr in lag:
                        xq_next[rr] = emit_quant(k + 1, rr,
                                                 clip_eng=nc.gpsimd)
                ps_list = [
                    psum_pool.tile([P, 2, 512], f32,
                                   name=f"ps{k}_{r}_{i}", tag="ps")
                    for i in range(2)
                ]
                if k == 0 and r == 0:
                    emit_conv_one(xq_half[0], 0, ps_list[0])
                    emit_conv_one(xq_half[1], 0, ps_list[1])
                else:
                    emit_conv_pair(xq_k[r], 2 * r, ps_list)
                for i in range(2):
                    j = 2 * r + i
                    g, itg = divmod(j, ITERS_PER_GROUP)
                    if itg == 0:
                        obs[g] = out_pool.tile([P, ITERS_PER_GROUP, 2, 512],
                                               bf16, name=f"ob{k}_{g}",
                                               tag="ob")
                    emit_epilogue(ps_list[i], obs[g], itg)
                    if itg == 3:
                        nc.sync.dma_start(yd[k, g, :, 0:4 * B2],
                                            obs[g][:, 0:4, :, 0:NFREE])
                    elif itg == 5:
                        nc.sync.dma_start(yd[k, g, :, 4 * B2:6 * B2],
                                            obs[g][:, 4:6, :, 0:NFREE])
                    elif itg == 6:
                        nc.sync.dma_start(yd[k, g, :, 6 * B2:7 * B2],
                                            obs[g][:, 6:7, :, 0:NFREE])
            xq_k = xq_next

    _split_multi_syncs(nc)
    nc.finalize()
    return nc


def _host_prep(w_q, s_exp, bias, act_exp):
    """Prescaled weights in lhsT layout (dup on both halves) + bias column."""
    s_exp = np.asarray(s_exp).reshape(-1).astype(np.float64)
    # matmul operands are real-valued clip(x) (not integer x/step), so only
    # the per-channel 2^s_exp factor goes into the weights; act_exp is
    # implicitly replayed by the bf16 grid of the operands.
    scale = np.exp2(s_exp)                                        # [64]
    wq = w_q.astype(np.float64) * scale.reshape(-1, 1, 1, 1)      # [co,ci,kh,kw]
    w_half = np.transpose(wq, (1, 2, 3, 0)).reshape(CIN, 9 * COUT)
    wsb = np.concatenate([w_half, w_half], axis=0).astype(ml_dtypes.bfloat16)

    col_bias = np.tile(np.asarray(bias, np.float32), 2).astype(np.float32)
    sb = col_bias.reshape(P, 1)                                   # [128, 1] f32
    return wsb, sb


def _stage_x(x):
    """f32 [32,64,112,112] -> fp16 zero-padded region layout
    [core, pair, region, 128, 18, 114] with 2-row halos duplicated."""
    xp = np.zeros((N_CORES * IMGS_PER_CORE, CIN, H + 2, WP), np.float16)
    xp[:, :, 1:1 + H, 1:1 + W] = x.astype(np.float16)
    xp = xp.reshape(N_CORES, N_PAIRS, 2, CIN, H + 2, WP)
    regs = [xp[:, :, :, :, 16 * r:16 * r + REGION_ROWS, :] for r in range(REGIONS)]
    xh = np.stack(regs, axis=2)   # core, pair, region, imgp, ch, row, col
    return np.ascontiguousarray(
        xh.reshape(N_CORES, N_PAIRS, REGIONS, P, REGION_ROWS, WP)
    )


def _assemble_y(yd_list):
    """Per-core bf16 [2,2,128,6272] device layout -> f32 [32,64,112,112]."""
    out = np.empty((N_CORES * IMGS_PER_CORE, COUT, H, W), np.float32)
    for c, yd in enumerate(yd_list):
        # dims: pair, g, half, ch, itg, ab, row, col
        v = np.asarray(yd).reshape(N_PAIRS, GROUPS, 2, 64, ITERS_PER_GROUP, 2,
                                   ROWS_PER_TILE, W).astype(np.float32)
        oc = out[IMGS_PER_CORE * c: IMGS_PER_CORE * (c + 1)]
        # out rows = 8*(7g+itg) + 4*blk + row -> (g, itg, blk, row) nesting
        o_r = oc.reshape(N_PAIRS, 2, 64, GROUPS, ITERS_PER_GROUP, 2,
                         ROWS_PER_TILE, W)
        for half in (0, 1):
            for ab in (0, 1):
                imgp, blk = half ^ ab, half
                o_r[:, imgp, :, :, :, blk] = \
                    v[:, :, half, :, :, ab].transpose(0, 2, 1, 3, 4, 5)
    return out


def _make_in_maps(x, w_q, s_exp, bias, act_exp):
    x = np.asarray(x, dtype=np.float32)
    wsb, sb = _host_prep(np.asarray(w_q), s_exp, bias, int(act_exp))
    xh = _stage_x(x)
    return [{"xh": xh[c], "wsb": wsb, "sb": sb} for c in range(N_CORES)]


def kernel(x, w_q, s_exp, bias, act_exp):
    from concourse.bass_utils import run_bass_kernel_spmd

    in_maps = _make_in_maps(x, w_q, s_exp, bias, act_exp)
    if "nc" not in _NC_CACHE:
        _NC_CACHE["nc"] = build_nc()
    nc = _NC_CACHE["nc"]

    res = run_bass_kernel_spmd(nc, in_maps, core_ids=list(range(N_CORES)))
    return _assemble_y([res.results[c]["yd"] for c in range(N_CORES)])


# revision 17
# speedup vs baseline: 1.1813x; 1.0140x over previous
"""BitConv2d (ternary-weight 3x3 conv, power-of-two rescale) on 8 TRN2 NeuronCores.

Strategy (v3):
  - Data-parallel over batch: 32 images -> 4 per core (2 image pairs).
  - Input staged to device as fp16 in a host-pretransposed, ZERO-PADDED
    layout [pair, region, 128, 18, 114] (partition = img_in_pair*64 + cin).
    Region r holds padded rows 16r..16r+17 (2-row halo duplicated on host),
    borders are zeros.  One contiguous 525 KB DMA per region.
  - Quantization replay: the reference computes xq = round(clip(x,±1)*64)/64.
    We use xv = bf16(clip(fp16(x),±1)) instead - the bf16 grid at |x|<=1 is
    finer (2^-8..2^-9) than the reference's quant step 2^-6, and the fp16
    staging error dominates; measured rel err ~4.6e-3 (budget 2e-2).  So the
    whole quant pipeline is ONE engine op per region: tensor_scalar
    min(x,1),max(x,-1) fp16->bf16, fully contiguous.  Zero borders clip to
    zero, contributing nothing: no bias folding needed.
  - Conv as 9 accumulating matmuls per output tile (K=Cin=64, M=Cout=64),
    packed 4-per-array with tile_position quadrants.  Per iteration (8 output
    rows) ONE two-bank psum tile [128, 896]:
      cols   0:448 <- quad (0,0)=img_a blk0 | quad (64,64)=img_b blk1
      cols 448:896 <- quad (64,0)=img_b blk0 | quad (0,64)=img_a blk1
    Taps loop over iteration PAIRS (both iters share the region tile) for
    deeper PE pipelining.
  - Weights are prescaled on host by 2^(act_exp+s_exp[co]) (exact: powers of
    two in bf16), so the epilogue is a single bias-add [128,896] psum->bf16,
    alternating ACT/DVE by iteration parity.
  - Output written bf16 in device-native layout, 8 DMAs of ~0.7-0.8 MB; host
    reassembles into f32 NCHW.
"""

import numpy as np
import ml_dtypes
from contextlib import ExitStack

_NC_CACHE = {}

N_CORES = 8
H = W = 112
WP = W + 2                    # padded cols
CIN = COUT = 64
P = 128
IMGS_PER_CORE = 4
N_PAIRS = 2
REGIONS = 7                   # input regions per pair
REGION_ROWS = 18              # padded rows per region (16 + 2 halo)
ROWS_PER_TILE = 4             # output rows per matmul tile
NFREE = ROWS_PER_TILE * W     # 448
N_ITERS = 14                  # conv iterations per pair (8 rows each)
GROUPS = 2                    # output buffers per pair
ITERS_PER_GROUP = 7


def _patch_tile_drain(tile_mod):
    """This walrus build rejects a Drain carrying many sync waits; split the
    final Tile drain into single-wait sync nops."""
    from concourse.vector_clock import ScopedClock, VectorClock

    if getattr(tile_mod.TileContext, "_drain_patched", False):
        return

    def _drain_and_barrier_split(self, tick_clock, wait_clock):
        vclock = tick_clock.global_clock
        n = len(vclock)
        for proc in range(n):
            t = vclock[proc]
            if t <= 0:
                continue
            vec = [0] * n
            vec[proc] = t
            nop = self.nc.sync.nop()
            wait_clock.add_sem_waits(nop.ins, ScopedClock({None: VectorClock(vec)}))
        self.nc.sync.drain()
        assert self.sems is not None
        popped = self.nc._tile_sem_poison_stack.pop()
        assert popped is self._sem_poison
        self.nc.all_engine_barrier()
        self.nc.clear_and_free_semaphores(list(self.sems.allocated().values()))
        self.nc.all_engine_barrier()

    tile_mod.TileContext._drain_and_barrier = _drain_and_barrier_split
    tile_mod.TileContext._drain_patched = True


def _split_multi_syncs(nc):
    """This walrus build accepts at most ONE sync wait (and one update) per
    instruction.  Hoist extra waits onto preceding nops and extra updates onto
    following nops (same engine, so ordering semantics are preserved)."""
    import concourse.mybir as mybir

    fn = nc.m.functions[0]
    ctr = 0
    for bb in fn.blocks:
        new_insts = []
        for inst in bb.instructions:
            si = inst.sync_info
            pre, post = [], []
            if si is not None and si.on_wait and len(si.on_wait) > 1:
                for w in list(si.on_wait[:-1]):
                    ctr += 1
                    pre.append(
                        mybir.InstNoOp(
                            name=f"wsplit_nop_{ctr}",
                            engine=inst.engine,
                            sync_info=mybir.SyncInfo(on_wait=[w], on_update=[]),
                        )
                    )
                si.on_wait = [si.on_wait[-1]]
            if (
                si is not None
                and si.on_update
                and len(si.on_update) > 1
                and not isinstance(inst, (mybir.InstDMACopy, mybir.InstDMA))
            ):
                for u in list(si.on_update[1:]):
                    ctr += 1
                    post.append(
                        mybir.InstNoOp(
                            name=f"usplit_nop_{ctr}",
                            engine=inst.engine,
                            sync_info=mybir.SyncInfo(on_wait=[], on_update=[u]),
                        )
                    )
                si.on_update = [si.on_update[0]]
            new_insts.extend(pre)
            new_insts.append(inst)
            new_insts.extend(post)
        if len(new_insts) != len(bb.instructions):
            bb.instructions[:] = new_insts
    for bb in fn.blocks:
        for inst in bb.instructions:
            if inst.name.startswith(("wsplit_nop_", "usplit_nop_")):
                if inst.name not in nc.inst_map:
                    nc.register_instruction(inst)
    return ctr


def build_nc():
    import concourse.bass as bass
    import concourse.mybir as mybir
    import concourse.tile as tile

    _patch_tile_drain(tile)

    f32 = mybir.dt.float32
    f16 = mybir.dt.float16
    bf16 = mybir.dt.bfloat16
    Alu = mybir.AluOpType
    Act = mybir.ActivationFunctionType

    nc = bass.Bass(trn_type="TRN2")
    xh = nc.dram_tensor(
        "xh", (N_PAIRS, REGIONS, P, REGION_ROWS, WP), f16, kind="ExternalInput"
    )
    wsb = nc.dram_tensor("wsb", (P, 9 * COUT), bf16, kind="ExternalInput")
    sb = nc.dram_tensor("sb", (P, 1), f32, kind="ExternalInput")
    yd = nc.dram_tensor(
        "yd", (N_PAIRS, GROUPS, P, ITERS_PER_GROUP * 2 * NFREE), bf16,
        kind="ExternalOutput",
    )

    with tile.TileContext(nc) as tc, ExitStack() as ctx:
        const_pool = ctx.enter_context(tc.tile_pool(name="const", bufs=1))
        xq_pool = ctx.enter_context(tc.tile_pool(name="xq", bufs=2 * REGIONS))
        stg_pool = ctx.enter_context(tc.tile_pool(name="stg", bufs=6))
        out_pool = ctx.enter_context(tc.tile_pool(name="out", bufs=3))
        psum_pool = ctx.enter_context(
            tc.tile_pool(name="psum", bufs=4, space=bass.MemorySpace.PSUM)
        )

        w_t = const_pool.tile([P, 9 * COUT], bf16)
        nc.sync.dma_start(w_t[:], wsb[:])
        sb_t = const_pool.tile([P, 1], f32)
        nc.sync.dma_start(sb_t[:], sb[:])

        def emit_quant(pair_idx, region, dma_eng=None, clip_eng=None):
            stg = stg_pool.tile([P, REGION_ROWS, WP], f16)
            (dma_eng or nc.sync).dma_start(stg[:], xh[pair_idx, region])
            xq = xq_pool.tile([P, REGION_ROWS, WP], bf16,
                              name=f"xq{pair_idx}_{region}", tag="xq")
            (clip_eng or nc.vector).tensor_scalar(
                out=xq[:], in0=stg[:], scalar1=1.0, scalar2=-1.0,
                op0=Alu.min, op1=Alu.max,
            )
            return xq

        def emit_conv_one(xq, b, ps):
            for tap in range(9):
                dh, dw = divmod(tap, 3)
                st, sp = tap == 0, tap == 8
                ws = slice(tap * 64, (tap + 1) * 64)
                for half, blk in ((0, 0), (1, 1), (1, 0), (0, 1)):
                    bank = half ^ blk
                    wsl = (slice(0, 64) if half == 0 else slice(64, 128))
                    psl = (slice(0, 64) if blk == 0 else slice(64, 128))
                    h = b + ROWS_PER_TILE * blk + dh
                    nc.tensor.matmul(
                        ps[psl, bank, 0:NFREE], w_t[wsl, ws],
                        xq[wsl, h:h + ROWS_PER_TILE, dw:dw + W],
                        start=st, stop=sp)

        def emit_conv_pair(xq, j0, ps_list):
            # two iterations j0, j0+1 share the region tile; loop taps
            # outermost so the PE queue sees 8 independent matmuls per tap.
            for tap in range(9):
                dh, dw = divmod(tap, 3)
                st, sp = tap == 0, tap == 8
                ws = slice(tap * 64, (tap + 1) * 64)
                for ps, b in zip(ps_list, (0, 8)):
                    for half, blk in ((0, 0), (1, 1), (1, 0), (0, 1)):
                        bank = half ^ blk
                        wsl = (slice(0, 64) if half == 0 else slice(64, 128))
                        psl = (slice(0, 64) if blk == 0 else slice(64, 128))
                        h = b + ROWS_PER_TILE * blk + dh
                        nc.tensor.matmul(
                            ps[psl, bank, 0:NFREE], w_t[wsl, ws],
                            xq[wsl, h:h + ROWS_PER_TILE, dw:dw + W],
                            start=st, stop=sp)

        def emit_epilogue(ps, ob, itg, split=False):
            if split:
                nc.scalar.activation(
                    ob[:, itg, 0], ps[:, 0], Act.Identity, bias=sb_t[:, 0:1],
                )
                nc.vector.tensor_scalar_add(ob[:, itg, 1], ps[:, 1],
                                            sb_t[:, 0:1])
            elif itg % 2 == 0:
                nc.scalar.activation(
                    ob[:, itg], ps[:], Act.Identity, bias=sb_t[:, 0:1],
                )
            else:
                nc.vector.tensor_scalar_add(ob[:, itg], ps[:], sb_t[:, 0:1])

        # software pipeline: conv(pair k) interleaves with quant(pair k+1).
        # Head: regions 0,1 staged on two different DMA queues (sync+scalar)
        # so both land early; remaining pair-0 clips are emitted interleaved
        # with conv so DVE's FIFO never head-of-line-blocks the epilogues.
        B2 = 2 * NFREE
        xq_k = [None] * REGIONS
        xq_half = []
        for hh, (r0, r1) in enumerate(((0, 10), (8, 18))):
            stgh = stg_pool.tile([P, 10, WP], f16, name=f"stgh{hh}", tag="stgh")
            nc.sync.dma_start(stgh[:], xh[0, 0, :, r0:r1, :])
            xqh = xq_pool.tile([P, 10, WP], bf16, name=f"xqh{hh}", tag="xqh")
            nc.vector.tensor_scalar(
                out=xqh[:], in0=stgh[:], scalar1=1.0, scalar2=-1.0,
                op0=Alu.min, op1=Alu.max,
            )
            xq_half.append(xqh)
        xq_k[1] = emit_quant(0, 1)
        xq_k[2] = emit_quant(0, 2)
        for r in range(3, REGIONS):
            xq_k[r] = emit_quant(0, r, clip_eng=nc.gpsimd)
        for k in range(N_PAIRS):
            xq_next = [None] * REGIONS
            obs = {}
            for r in range(REGIONS):
                if k + 1 < N_PAIRS:
                    lag = [r - 2] if 0 <= r - 2 else []
                    if r == REGIONS - 1:
                        lag = [r - 2, r - 1, r]
                    for rr in lag:
                        xq_next[rr] = emit_quant(k + 1, rr,
                                                 clip_eng=nc.gpsimd)
                ps_list = [
                    psum_pool.tile([P, 2, 512], f32,
                                   name=f"ps{k}_{r}_{i}", tag="ps")
                    for i in range(2)
                ]
                if k == 0 and r == 0:
                    emit_conv_one(xq_half[0], 0, ps_list[0])
                    emit_conv_one(xq_half[1], 0, ps_list[1])
                else:
                    emit_conv_pair(xq_k[r], 2 * r, ps_list)
                for i in range(2):
                    j = 2 * r + i
                    g, itg = divmod(j, ITERS_PER_GROUP)
                    if itg == 0:
                        obs[g] = out_pool.tile([P, ITERS_PER_GROUP, 2, 512],
                                               bf16, name=f"ob{k}_{g}",
                                               tag="ob")
                    last = (k == N_PAIRS - 1 and g == GROUPS - 1)
                    emit_epilogue(ps_list[i], obs[g], itg,
                                  split=last and itg >= 5)
                    if itg == 3:
                        nc.sync.dma_start(yd[k, g, :, 0:4 * B2],
                                            obs[g][:, 0:4, :, 0:NFREE])
                    elif itg == 5:
                        nc.sync.dma_start(yd[k, g, :, 4 * B2:6 * B2],
                                            obs[g][:, 4:6, :, 0:NFREE])
                    elif itg == 6:
                        eng = nc.scalar if last else nc.sync
                        eng.dma_start(yd[k, g, :, 6 * B2:7 * B2],
                                      obs[g][:, 6:7, :, 0:NFREE])
            xq_k = xq_next

    _split_multi_syncs(nc)
    nc.finalize()
    return nc


def _host_prep(w_q, s_exp, bias, act_exp):
    """Prescaled weights in lhsT layout (dup on both halves) + bias column."""
    s_exp = np.asarray(s_exp).reshape(-1).astype(np.float64)
    # matmul operands are real-valued clip(x) (not integer x/step), so only
    # the per-channel 2^s_exp factor goes into the weights; act_exp is
    # implicitly replayed by the bf16 grid of the operands.
    scale = np.exp2(s_exp)                                        # [64]
    wq = w_q.astype(np.float64) * scale.reshape(-1, 1, 1, 1)      # [co,ci,kh,kw]
    w_half = np.transpose(wq, (1, 2, 3, 0)).reshape(CIN, 9 * COUT)
    wsb = np.concatenate([w_half, w_half], axis=0).astype(ml_dtypes.bfloat16)

    col_bias = np.tile(np.asarray(bias, np.float32), 2).astype(np.float32)
    sb = col_bias.reshape(P, 1)                                   # [128, 1] f32
    return wsb, sb


def _stage_x(x):
    """f32 [32,64,112,112] -> fp16 zero-padded region layout
    [core, pair, region, 128, 18, 114] with 2-row halos duplicated."""
    xp = np.zeros((N_CORES * IMGS_PER_CORE, CIN, H + 2, WP), np.float16)
    xp[:, :, 1:1 + H, 1:1 + W] = x.astype(np.float16)
    xp = xp.reshape(N_CORES, N_PAIRS, 2, CIN, H + 2, WP)
    regs = [xp[:, :, :, :, 16 * r:16 * r + REGION_ROWS, :] for r in range(REGIONS)]
    xh = np.stack(regs, axis=2)   # core, pair, region, imgp, ch, row, col
    return np.ascontiguousarray(
        xh.reshape(N_CORES, N_PAIRS, REGIONS, P, REGION_ROWS, WP)
    )


def _assemble_y(yd_list):
    """Per-core bf16 [2,2,128,6272] device layout -> f32 [32,64,112,112]."""
    out = np.empty((N_CORES * IMGS_PER_CORE, COUT, H, W), np.float32)
    for c, yd in enumerate(yd_list):
        # dims: pair, g, half, ch, itg, ab, row, col
        v = np.asarray(yd).reshape(N_PAIRS, GROUPS, 2, 64, ITERS_PER_GROUP, 2,
                                   ROWS_PER_TILE, W).astype(np.float32)
        oc = out[IMGS_PER_CORE * c: IMGS_PER_CORE * (c + 1)]
        # out rows = 8*(7g+itg) + 4*blk + row -> (g, itg, blk, row) nesting
        o_r = oc.reshape(N_PAIRS, 2, 64, GROUPS, ITERS_PER_GROUP, 2,
                         ROWS_PER_TILE, W)
        for half in (0, 1):
            for ab in (0, 1):
                imgp, blk = half ^ ab, half
                o_r[:, imgp, :, :, :, blk] = \
                    v[:, :, half, :, :, ab].transpose(0, 2, 1, 3, 4, 5)
    return out


def _make_in_maps(x, w_q, s_exp, bias, act_exp):
    x = np.asarray(x, dtype=np.float32)
    wsb, sb = _host_prep(np.asarray(w_q), s_exp, bias, int(act_exp))
    xh = _stage_x(x)
    return [{"xh": xh[c], "wsb": wsb, "sb": sb} for c in range(N_CORES)]


def kernel(x, w_q, s_exp, bias, act_exp):
    from concourse.bass_utils import run_bass_kernel_spmd

    in_maps = _make_in_maps(x, w_q, s_exp, bias, act_exp)
    if "nc" not in _NC_CACHE:
        _NC_CACHE["nc"] = build_nc()
    nc = _NC_CACHE["nc"]

    res = run_bass_kernel_spmd(nc, in_maps, core_ids=list(range(N_CORES)))
    return _assemble_y([res.results[c]["yd"] for c in range(N_CORES)])
